# revision 1
# baseline (speedup 1.0000x reference)
"""Trainium2 Bass kernel for causal single-head attention (B=16, S=2048, D=64).

Sharding: data-parallel over batch. 8 NeuronCores, 2 batches per core.

v3: software-pipelined flat iteration stream, PE kept continuously busy.
  - exp split: ACT true Exp (~75% of tiles) / DVE Schraudolph bf16 exp
    (~25%): et_i16 = int16(score*23.0825 + 16251.8); bitcast bf16 IS
    exp(score*0.125) to ~3%: sawtooth error cancels in softmax for this
    problem's tiny-score regime.
  - causal mask: PE-internal. Seed matmuls write a -BIG lower triangle
    (via identity @ triC const) into the diagonal 128-col region with
    start=True; the score matmul accumulates onto it with start=False.
    exp(-BIG)=0 / schraudolph(-BIG)=bf16-denormal=0. No elementwise
    engine in the mask path.
  - PSUM: 3-slot score ring (6 banks) + 2 accumulator banks. Prologue
    transposes/projections and epilogue Wv outputs share the score ring.
  - All PSUM->SBUF copies (q/k proj, xT, acc, divide) on DVE; gpsimd
    only does the f32->bf16 input cast; prologue/epilogue interleaved.
"""

import numpy as np
from contextlib import ExitStack

NB = 2  # batches per core
S = 2048
D = 64
P = 128
NT = S // P
W = 512
NCH = S // W
KPC = W // P
N_CORES = 8

_CACHE = {}

_N_ITERS = sum(KPC * (c + 1) for c in range(NCH))  # 40
SCH_ALPHA = 128.0 / np.log(2.0)  # 184.664
SCH_BETA = 16251.8
SEED_ACT = -30000.0
SEED_DVE = -700.0


def _build_nc():
    import concourse.bass as bass
    import concourse.tile as tile
    from concourse import bacc, mybir
    from concourse.masks import make_identity

    f32 = mybir.dt.float32
    bf16 = mybir.dt.bfloat16
    i16 = mybir.dt.int16
    AF = mybir.ActivationFunctionType
    ALU = mybir.AluOpType

    nc = bacc.Bacc(None, target_bir_lowering=False, debug=False)

    x_ext = nc.declare_dram_parameter("x", [NB, S, D], f32, isOutput=False)
    w_ext = {}
    for wname in ("Wq", "Wk", "Wv"):
        w_ext[wname] = nc.declare_dram_parameter(wname, [D, D], f32, isOutput=False)
    for bname in ("bq", "bk", "bv"):
        w_ext[bname] = nc.declare_dram_parameter(bname, [D], f32, isOutput=False)
    out_ext = nc.declare_dram_parameter("out", [NB, S, D], f32, isOutput=True)

    # DVE (Schraudolph) handles every 4th iteration
    use_act = [gi % 4 != 3 for gi in range(_N_ITERS)]

    with ExitStack() as ctx:
        tc = ctx.enter_context(tile.TileContext(nc))

        singles = ctx.enter_context(tc.tile_pool(name="singles", bufs=1))
        xstage = ctx.enter_context(tc.tile_pool(name="xstage", bufs=3))
        etp = ctx.enter_context(tc.tile_pool(name="etp", bufs=8))
        outst = ctx.enter_context(tc.tile_pool(name="outst", bufs=3))
        scp = ctx.enter_context(
            tc.tile_pool(name="scp", bufs=3, space=bass.MemorySpace.PSUM)
        )
        accp = ctx.enter_context(
            tc.tile_pool(name="accp", bufs=1, space=bass.MemorySpace.PSUM)
        )

        def stage_x_dma(c):
            tiles = []
            rows4 = bass.ds(W * c, W)
            for b in range(NB):
                xf = xstage.tile([P, KPC, D], f32, tag="xf32")
                nc.sync.dma_start(
                    out=xf,
                    in_=x_ext.ap()[b, rows4, :].rearrange("(t p) d -> p t d", p=P),
                )
                tiles.append(xf)
            return tiles

        # chunk-0 x DMA first: it gates the whole prologue chain
        xf_staged = {0: stage_x_dma(0)}

        # ---- constants ----
        ident = singles.tile([P, P], bf16)
        make_identity(nc, ident)

        # strict lower-triangle seed constants (tri[r, j] = seed if j < r)
        tri = {}
        for nm, seed in (("act", SEED_ACT), ("dve", SEED_DVE)):
            t = singles.tile([P, P], bf16, name=f"tri_{nm}")
            nc.gpsimd.memset(t, 0.0)
            nc.gpsimd.affine_select(
                out=t, in_=t, base=0, channel_multiplier=-1,
                pattern=[[1, P]], compare_op=ALU.is_ge, fill=seed,
            )
            tri[nm] = t

        # PE p-state warmup: ~24 dummy transposes (no data deps beyond ident)
        warm = scp.tile([P, P], bf16, tag="sc")
        for _ in range(24):
            nc.tensor.transpose(warm, ident, ident)

        w_aug = {}
        for wname, bname in (("Wq", "bq"), ("Wk", "bk"), ("Wv", "bv")):
            aug = singles.tile([D + 1, D], bf16, name=f"{wname}_aug")
            wtmp = xstage.tile([D, D], f32, tag="wtmp")
            btmp = xstage.tile([1, D], f32, tag="btmp")
            nc.sync.dma_start(out=wtmp, in_=w_ext[wname].ap())
            nc.sync.dma_start(
                out=btmp, in_=w_ext[bname].ap().rearrange("(a d) -> a d", a=1)
            )
            nc.vector.tensor_copy(out=aug[0:D, :], in_=wtmp)
            nc.vector.tensor_copy(out=aug[D : D + 1, :], in_=btmp)
            w_aug[wname] = aug

        # ---- persistent tiles ----
        x_bf = []
        xT_aug = []
        for b in range(NB):
            x_bf.append(singles.tile([P, NT, D + 1], bf16, name=f"x_bf{b}"))
            xT_aug.append(singles.tile([P, S], bf16, name=f"xT_aug{b}"))
        qT_all = singles.tile([P, S], bf16)
        kT_all = singles.tile([P, S], bf16)
        acc_sbuf = [
            singles.tile([D + 1, NCH, W], bf16, name=f"acc_sbuf{b}") for b in range(NB)
        ]
        rowsum_resh = [
            singles.tile([KPC, NCH, P], bf16, name=f"rowsum_resh{b}")
            for b in range(NB)
        ]
        recip_all = [singles.tile([P, NT], f32, name=f"recip{b}") for b in range(NB)]

        def prologue_cast(c, xf_tiles, eng=None):
            eng = eng or nc.gpsimd
            ts4 = slice(KPC * c, KPC * (c + 1))
            for b in range(NB):
                eng.tensor_copy(out=x_bf[b][:, ts4, 0:D], in_=xf_tiles[b])
                eng.memset(x_bf[b][:, ts4, D : D + 1], 1.0)

        def prologue_transpose_half(c, b, pt):
            rows4 = bass.ds(W * c, W)
            for tt in range(KPC):
                nc.tensor.transpose(
                    pt[:, KPC * b + tt, :], x_bf[b][:, KPC * c + tt, :], ident
                )
            nc.vector.tensor_copy(
                out=xT_aug[b][0 : D + 1, rows4],
                in_=pt[:, KPC * b : KPC * (b + 1), :].rearrange("e t p -> e (t p)"),
            )

        def prologue_transpose(c):
            pt = scp.tile([D + 1, 2 * KPC, P], bf16, tag="sc")
            for b in range(NB):
                prologue_transpose_half(c, b, pt)

        def prologue_proj_half(c, b, qk):
            rows4 = bass.ds(W * c, W)
            pr = bass.ds(b * D, D)
            nc.tensor.matmul(
                qk[pr, 0, :], w_aug["Wq"], xT_aug[b][0 : D + 1, rows4],
                tile_position=(0, b * D),
            )
            nc.tensor.matmul(
                qk[pr, 1, :], w_aug["Wk"], xT_aug[b][0 : D + 1, rows4],
                tile_position=(0, b * D),
            )

        def prologue_proj_copies(c, qk):
            rows4 = bass.ds(W * c, W)
            nc.vector.tensor_copy(out=qT_all[:, rows4], in_=qk[:, 0, :])
            nc.vector.tensor_copy(out=kT_all[:, rows4], in_=qk[:, 1, :])

        def prologue_proj(c):
            qk = scp.tile([P, 2, W], f32, tag="sc")
            for b in range(NB):
                prologue_proj_half(c, b, qk)
            prologue_proj_copies(c, qk)

        xf_staged[1] = stage_x_dma(1)

        acc = [None, None]
        pending_av = []
        hook_state = {}

        def emit_score(c, i, gi):
            off0 = max(0, P * i - W * c)
            span = W - off0
            q0 = W * c + off0
            diag = i >= KPC * c
            sc = scp.tile([P, 2 * W], f32, tag="sc")
            trc = tri["act"] if use_act[gi] else tri["dve"]
            if diag:
                for b in range(NB):
                    reg0 = off0 if b == 0 else W
                    nc.tensor.matmul(
                        sc[:, reg0 : reg0 + P], ident, trc, start=True, stop=False,
                        skip_group_check=True,
                    )
            for b in range(NB):
                rows = bass.ds(b * D, D)
                ktile = kT_all[rows, bass.ds(P * i, P)]
                if diag:
                    reg0 = off0 if b == 0 else W
                    nc.tensor.matmul(
                        sc[:, reg0 : reg0 + P], ktile,
                        qT_all[rows, bass.ds(q0, P)],
                        start=False, stop=True, skip_group_check=True,
                    )
                    if span > P:
                        nc.tensor.matmul(
                            sc[:, reg0 + P : reg0 + span], ktile,
                            qT_all[rows, bass.ds(q0 + P, span - P)],
                        )
                else:
                    dst = sc[:, off0:W] if b == 0 else sc[:, W : W + span]
                    nc.tensor.matmul(dst, ktile, qT_all[rows, bass.ds(q0, span)])
            return sc, off0, span

        def emit_exp(c, i, gi, sc, off0, span):
            reg = slice(off0, W + span)
            if use_act[gi]:
                et = etp.tile([P, 2 * W], bf16, tag="et")
                nc.scalar.activation(
                    out=et[:, reg], in_=sc[:, reg], func=AF.Exp, scale=0.125
                )
                return et, et
            et = etp.tile([P, 2 * W], i16, tag="et")
            nc.vector.tensor_scalar(
                out=et[:, reg], in0=sc[:, reg],
                scalar1=float(SCH_ALPHA * 0.125), scalar2=float(SCH_BETA),
                op0=ALU.mult, op1=ALU.add,
            )
            return et, et.bitcast(bf16)

        def flush_av(upto_gi):
            while pending_av and pending_av[0][0] <= upto_gi:
                _, c, i, etb, nk = pending_av.pop(0)
                off0 = max(0, P * i - W * c)
                span = W - off0
                for b in range(NB):
                    regb = etb[:, off0:W] if b == 0 else etb[:, W : W + span]
                    nc.tensor.matmul(
                        acc[b][:, off0:W], x_bf[b][:, i, :], regb,
                        start=(i == 0), stop=(i == nk - 1),
                    )

        def epilogue_a(c):
            """acc -> SBUF (split ACT/DVE) + rowsum extraction DMAs."""
            nc.scalar.copy(out=acc_sbuf[0][:, c, :], in_=acc[0])
            nc.vector.tensor_copy(out=acc_sbuf[1][:, c, :], in_=acc[1])
            for b in range(NB):
                nc.sync.dma_start(
                    out=rowsum_resh[b][:, c, :],
                    in_=acc_sbuf[b][D : D + 1, c, :],
                )

        def epilogue_b(c):
            po = scp.tile([P, 2 * KPC * D], f32, tag="sc")
            rst = scp.tile([P, 2 * KPC], bf16, tag="sc")
            for b in range(NB):
                for j in range(KPC):
                    nc.tensor.matmul(
                        po[:, bass.ds(b * KPC * D + j * D, D)],
                        acc_sbuf[b][:, c, bass.ds(P * j, P)],
                        w_aug["Wv"],
                    )
                nc.tensor.transpose(
                    rst[:, bass.ds(b * KPC, KPC)],
                    rowsum_resh[b][:, c, :],
                    ident[0:KPC, 0:KPC],
                )
                nc.vector.reciprocal(
                    out=recip_all[b][:, bass.ds(KPC * c, KPC)],
                    in_=rst[:, bass.ds(b * KPC, KPC)],
                )
            nways = 2 if c == NCH - 1 else 1
            jr = KPC // nways
            for h in range(nways):
                for b in range(NB):
                    div = outst.tile([P, jr, D], f32, tag="div")
                    rc = recip_all[b][:, KPC * c + h * jr : KPC * c + (h + 1) * jr]
                    rc_b = bass.AP(
                        tensor=rc.tensor, offset=rc.offset,
                        ap=[rc.ap[0], rc.ap[1], [0, D]],
                    )
                    pob = po[
                        :, bass.ds(b * KPC * D + h * jr * D, jr * D)
                    ].rearrange("p (j d) -> p j d", j=jr)
                    nc.vector.tensor_mul(div, pob, rc_b)
                    nc.sync.dma_start(
                        out=out_ext.ap()[
                            b, bass.ds(W * c + h * jr * P, jr * P), :
                        ].rearrange("(j p) d -> p j d", p=P),
                        in_=div,
                    )

        # ---------- main schedule ----------
        prologue_cast(0, xf_staged.pop(0), eng=nc.vector)
        prologue_transpose(0)
        prologue_proj(0)

        gi = 0
        for c in range(NCH):
            nk = KPC * c + KPC
            if c + 2 < NCH:
                xf_staged[c + 2] = stage_x_dma(c + 2)
            acc[0] = accp.tile([D + 1, W], f32, name=f"avacc0_{c}", tag="avacc0")
            acc[1] = accp.tile([D + 1, W], f32, name=f"avacc1_{c}", tag="avacc1")
            for i in range(nk):
                sc, off0, span = emit_score(c, i, gi)
                _, etb = emit_exp(c, i, gi, sc, off0, span)
                lag = 2 if i > 1 else 3
                pending_av.append((gi + lag, c, i, etb, nk))
                flush_av(gi)
                if c + 1 < NCH:
                    if i == 1:
                        prologue_cast(c + 1, xf_staged.pop(c + 1))
                    elif i == 2:
                        prologue_transpose(c + 1)
                    elif i == 3:
                        prologue_proj(c + 1)
                if c > 0 and i == 2:
                    epilogue_a(c - 1)
                if c > 0 and i == min(6, nk - 2):
                    epilogue_b(c - 1)
                gi += 1
        flush_av(gi + 10)
        epilogue_a(NCH - 1)
        epilogue_b(NCH - 1)

    nc.compile()
    return nc


def _get_nc():
    if "nc" not in _CACHE:
        _CACHE["nc"] = _build_nc()
    return _CACHE["nc"]


def kernel(**inputs) -> np.ndarray:
    from concourse.bass_utils import run_bass_kernel_spmd

    nc = _get_nc()
    x = np.ascontiguousarray(inputs["x"], dtype=np.float32)
    B = x.shape[0]
    assert B == NB * N_CORES
    reps = {
        k: np.ascontiguousarray(inputs[k], dtype=np.float32)
        for k in ("Wq", "bq", "Wk", "bk", "Wv", "bv")
    }
    in_maps = [
        {"x": np.ascontiguousarray(x[i * NB : (i + 1) * NB]), **reps}
        for i in range(N_CORES)
    ]
    res = run_bass_kernel_spmd(nc, in_maps, core_ids=list(range(N_CORES)))
    out = np.concatenate([res.results[i]["out"] for i in range(N_CORES)], axis=0)
    return out.astype(np.float32)

